# revision 2
# baseline (speedup 1.0000x reference)
"""Conv2d 3x3 (stride 1, pad 1) forward, data-parallel over batch on 8 trn2 cores.

x (16,64,224,224) * w (128,64,3,3) + b (128,) -> (16,128,224,224), fp32 in/out,
bf16 on-device compute (fp32 PSUM accumulate).

Per core (2 images): rows are split into even/odd phases on the SBUF partition
axis (p = phase*64 + ci). 6 of the 9 conv taps run as K=128 bf16 matmuls (pairs
of kh taps sharing a row offset); the remaining 3 taps per phase run as K=64
matmuls on complementary partition halves (concurrent on the PE array).

vs v1: bf16 everywhere (halves DMA + enables FWL weight loads), host-side
pre-packing so every input DMA is one contiguous [128, 4104] copy (no memsets,
no strided descriptors), phase-split output DRAM layout so output DMAs are
contiguous 7KB-per-partition writes, and matmuls grouped 4-wide per weight
(8 PSUM banks) so each weight slot serves 4 consecutive matmuls.
"""

import sys

sys.path.insert(0, "/opt/trn_rl_repo")

import numpy as np
import ml_dtypes

import concourse.bass as bass
import concourse.mybir as mybir
import concourse.tile as tile
from concourse import bacc
from concourse.bass_utils import run_bass_kernel_spmd

N_FULL, C_IN, H, W = 16, 64, 224, 224
C_OUT = 128
N_CORES = 8
N_PER = N_FULL // N_CORES  # 2 images per core

U = 16  # row-pairs of output per block (32 output rows)
SLOTS = U + 2
SW = 228  # padded row width: [pad, 224 data, pad, pad, pad]
NB = (H // 2) // U  # 7 blocks per image
NBLK = N_PER * NB  # 14 blocks per core
G = 4  # matmuls per weight slot (psum tiles per parity)
F32 = mybir.dt.float32
BF16 = mybir.dt.bfloat16
BF16NP = ml_dtypes.bfloat16

_CACHE = {}


def _build_nc(loop_reps=0):
    nc = bacc.Bacc("TRN2", target_bir_lowering=False, debug=False)
    x_t = nc.dram_tensor("x", [NBLK, 128, SLOTS * SW], BF16, kind="ExternalInput")
    wt_t = nc.dram_tensor("wt", [128, 9, 128], BF16, kind="ExternalInput")
    b_t = nc.dram_tensor("bias", [C_OUT], F32, kind="ExternalInput")
    # phase-split output: out[n, ph, co, u, w] = y[n, co, 2u+ph, w]
    o_t = nc.dram_tensor("out", [N_PER, 2, C_OUT, H // 2, W], BF16,
                         kind="ExternalOutput")
    x_ap, wt_ap, b_ap, o_ap = x_t.ap(), wt_t.ap(), b_t.ap(), o_t.ap()

    with tile.TileContext(nc) as tc:
        with (
            tc.tile_pool(name="const", bufs=1) as cpool,
            tc.tile_pool(name="outp", bufs=2) as opool,
            tc.tile_pool(name="psum", bufs=1, space="PSUM") as ppool,
        ):
            wsb = cpool.tile([128, 9, 128], BF16)
            nc.sync.dma_start(wsb[:], wt_ap)
            bias_sb = cpool.tile([128, 1], F32)
            nc.sync.dma_start(bias_sb[:], b_ap[:, None])

            xbufs = [
                cpool.tile([128, SLOTS * SW + 4], BF16, name=f"xb{i}")
                for i in range(2)
            ]
            for xb in xbufs:
                nc.vector.memset(xb[:], 0.0)

            # 8 persistent PSUM tiles = all 8 banks; reused every group with
            # address-tracked WAR deps (activation read -> next group's start)
            pes = [ppool.tile([128, 456], F32, name=f"pe{g}") for g in range(G)]
            pos = [ppool.tile([128, 456], F32, name=f"po{g}") for g in range(G)]

            import contextlib
            loop_cm = tc.For_i(0, loop_reps, 1) if loop_reps else contextlib.nullcontext()
            with loop_cm:
             for n in range(N_PER):
              for b in range(NB):
                blk = n * NB + b
                u0 = b * U
                xt = xbufs[blk % 2]
                nc.sync.dma_start(xt[:, : SLOTS * SW], x_ap[blk])

                obe = opool.tile([128, U, 224], BF16, name="obe")
                obo = opool.tile([128, U, 224], BF16, name="obo")

                for g0 in range(0, U // 2, G):
                    # s0[g]: first slot read by group member g (2 slots wide)
                    s0s = [2 * (g0 + g) + 1 for g in range(G)]
                    # even-phase row-pairs: kh=1 on even rows + kh=2 on odd (K=128)
                    for kw in range(3):
                        for g in range(G):
                            off = s0s[g] * SW + kw
                            nc.tensor.matmul(
                                pes[g][:], wsb[:, kw, :], xt[:, off : off + 456],
                                start=(kw == 0), stop=False,
                            )
                    # odd-phase row-pairs: kh=0 on even rows + kh=1 on odd (K=128)
                    for kw in range(3):
                        for g in range(G):
                            off = s0s[g] * SW + kw
                            nc.tensor.matmul(
                                pos[g][:], wsb[:, 3 + kw, :], xt[:, off : off + 456],
                                start=(kw == 0), stop=False,
                            )
                    # K=64 singles, strictly alternating complementary row
                    # halves (pe upper / po lower) so consecutive MMs run
                    # concurrently on disjoint PE row groups and each implicit
                    # LDWEIGHTS overlaps the other half's in-flight MM:
                    # even outputs need kh=0 from odd rows of the previous slot;
                    # odd outputs need kh=2 from even rows of the next slot.
                    for kw in range(3):
                        for g in range(G):
                            offe = (s0s[g] - 1) * SW + kw
                            offo = (s0s[g] + 1) * SW + kw
                            nc.tensor.matmul(
                                pes[g][:], wsb[64:128, 6 + kw, :],
                                xt[64:128, offe : offe + 456],
                                start=False, stop=(kw == 2),
                            )
                            nc.tensor.matmul(
                                pos[g][:], wsb[0:64, 6 + kw, :],
                                xt[0:64, offo : offo + 456],
                                start=False, stop=(kw == 2),
                            )
                    for g in range(G):
                        j0 = 2 * (g0 + g)
                        pev = pes[g][:].rearrange("p (r c) -> p r c", c=SW)
                        pov = pos[g][:].rearrange("p (r c) -> p r c", c=SW)
                        nc.scalar.activation(
                            obe[:, j0 : j0 + 2, :], pev[:, :, 0:224],
                            mybir.ActivationFunctionType.Identity, bias=bias_sb[:],
                        )
                        nc.scalar.activation(
                            obo[:, j0 : j0 + 2, :], pov[:, :, 0:224],
                            mybir.ActivationFunctionType.Identity, bias=bias_sb[:],
                        )

                nc.sync.dma_start(o_ap[n, 0, :, u0 : u0 + U, :], obe[:])
                nc.sync.dma_start(o_ap[n, 1, :, u0 : u0 + U, :], obo[:])

    nc.finalize()
    return nc


def _pack_weights(weight):
    wt = np.empty((128, 9, 128), dtype=np.float32)
    for kw in range(3):
        wt[:64, kw, :] = weight[:, :, 1, kw].T
        wt[64:, kw, :] = weight[:, :, 2, kw].T
        wt[:64, 3 + kw, :] = weight[:, :, 0, kw].T
        wt[64:, 3 + kw, :] = weight[:, :, 1, kw].T
        wt[:64, 6 + kw, :] = weight[:, :, 2, kw].T
        wt[64:, 6 + kw, :] = weight[:, :, 0, kw].T
    return wt.astype(BF16NP)


def _pack_x(x):
    """x [N,64,224,224] fp32 -> [N, NB, 128, SLOTS, SW] bf16, phase-split rows
    with one-column left pad and halo rows included per block."""
    n_img = x.shape[0]
    xb = x.astype(BF16NP)
    xp = np.zeros((n_img, NB, 128, SLOTS, SW), dtype=BF16NP)
    for b in range(NB):
        for r in range(SLOTS):
            pr = b * U - 1 + r  # row-pair index held by slot r
            if pr < 0 or pr >= H // 2:
                continue
            xp[:, b, 0:64, r, 1:225] = xb[:, :, 2 * pr, :]
            xp[:, b, 64:128, r, 1:225] = xb[:, :, 2 * pr + 1, :]
    return xp


def kernel(x, weight, bias, _trace=False):
    x = np.asarray(x, dtype=np.float32)
    weight = np.asarray(weight, dtype=np.float32)
    bias = np.ascontiguousarray(np.asarray(bias, dtype=np.float32))
    wt = np.ascontiguousarray(_pack_weights(weight))
    xp = _pack_x(x)  # [16, NB, 128, SLOTS, SW]

    if "nc" not in _CACHE:
        _CACHE["nc"] = _build_nc()
    nc = _CACHE["nc"]

    in_maps = [
        {
            "x": np.ascontiguousarray(
                xp[c * N_PER : (c + 1) * N_PER].reshape(NBLK, 128, SLOTS * SW)
            ),
            "wt": wt,
            "bias": bias,
        }
        for c in range(N_CORES)
    ]
    res = run_bass_kernel_spmd(
        nc, in_maps, core_ids=list(range(N_CORES)), trace=_trace
    )
    out = np.empty((N_FULL, C_OUT, H, W), dtype=np.float32)
    for c in range(N_CORES):
        oph = np.asarray(res.results[c]["out"]).astype(np.float32)
        out[c * N_PER : (c + 1) * N_PER, :, 0::2, :] = oph[:, 0]
        out[c * N_PER : (c + 1) * N_PER, :, 1::2, :] = oph[:, 1]
    if _trace:
        _CACHE["last_result"] = res
    return out
